# revision 1
# baseline (speedup 1.0000x reference)
"""Trainium2 Bass kernel for nn_MinimumSpanningTree.

Contract: kernel(**inputs) takes the FULL inputs (guide_in [8, 64, 256, 256]
f32) and returns the FULL output (tree [8, 65535, 2] int32).

Strategy (data-parallel over batch, one image per NeuronCore):
  Device (Bass, 8 cores SPMD) computes the memory-bound edge-weight build
  via the algebraic split  w(p,q) = S(p) + S(q) - 2*D(p,q) + 1  where
  S(p) = sum_c x[c,p]^2 and D(p,q) = sum_c x[c,p] x[c,q]:
    - 3 elementwise passes (sq = x*x on ACT, prodrow = x * x(+256) and
      prodcol = x * x(+1) split across DVE/Pool by columns), each writing
      float32r (TRN2 PE reduced precision: RNE to 11 mantissa bits; final
      tree rel-err vs exact ~5e-3, validated by simulation).
    - channel reduction on the PE: fp32r ones-matmuls (1 cycle/row) with a
      sliding pair-ones stationary window; PSUM-accumulates packed
      [128,512] S / Drow / Dcol banks laid out part = pixel//512.
    - product bodies cover [0, 3840) columns fully in-tile; each chunk
      boundary's 512-pixel block is reduced by a pair of 256-column
      matmuls (first half from the previous chunk's body, second half
      from a tiny head product), so no tile is written across
      iterations and the PE never stalls on a future chunk. The first
      chunk is quartered and the last chunk halved to shrink the
      pipeline fill and drain.
    - f32 combines w = (S+1)+S' - 2D on DVE; the odd-row S shift comes
      from a small partition-shift SBUF DMA that gates only the final
      single add.
  Boruvka MST per image (exactly the reference algorithm) + output
  assembly run on host.

Self-contained: shapes/sharding hardcoded.
"""
import numpy as np

B, C, H, W = 8, 64, 256, 256
V = H * W
E_ROW = (H - 1) * W
E_COL = H * (W - 1)
E = E_ROW + E_COL
N_ROUNDS = 16

HALF = V // 2          # 32768 pixels per partition-half
CH = 4096              # pixels per chunk (per half)
NCH = HALF // CH       # 8 chunk-pairs
NBLK = CH // 512       # 512-px matmul blocks per chunk
BODY = CH - 512        # 3584 body columns
POOL_COLS = 2848       # columns of each prodrow body computed on Pool

_compiled = None


def _build_program():
    import concourse.bacc as bacc
    import concourse.mybir as mybir
    from concourse import tile

    F32 = mybir.dt.float32
    F32R = mybir.dt.float32r
    AL = mybir.AluOpType
    ACT = mybir.ActivationFunctionType

    nc = bacc.Bacc('TRN2', target_bir_lowering=False, debug=False, num_devices=8)
    d_fm = nc.dram_tensor("fm", [C, V], F32, kind="ExternalInput")
    o_wrow = nc.dram_tensor("wrow", [128, 512], F32, kind="ExternalOutput")
    o_wcol = nc.dram_tensor("wcol", [128, 512], F32, kind="ExternalOutput")

    PRB_W = CH - 256   # 3840: prodrow body width (all in-tile)
    PCB_W = CH - 256   # 3840: prodcol body width
    QB = CH // 4       # 1024
    HB = CH // 2       # 2048

    with tile.TileContext(nc) as tc:
        with tc.tile_pool(name="inp", bufs=3) as inp, \
             tc.tile_pool(name="bod", bufs=2) as bod, \
             tc.tile_pool(name="hed", bufs=2) as hed, \
             tc.tile_pool(name="cst", bufs=1) as cst, \
             tc.tile_pool(name="fin", bufs=1) as fin, \
             tc.tile_pool(name="ps", bufs=1, space="PSUM") as psum:

            # sliding pair-ones stationary: col 63 = ones@0:64, col 127 =
            # ones@64:128; window [:, 63-u : 191-u] puts them at stationary
            # columns u and 64+u -> the matmul writes partitions (u, 64+u).
            buf_f = cst.tile([128, 192], F32)
            nc.gpsimd.memset(buf_f[:], 0.0)
            nc.gpsimd.memset(buf_f[0:64, 63:64], 1.0)
            nc.gpsimd.memset(buf_f[64:128, 127:128], 1.0)
            stat = cst.tile([128, 192], F32R)
            nc.gpsimd.tensor_scalar_mul(stat[:], buf_f[:], 1.0)
            # dummy activation: pulls the ACT table load off the critical path
            scratch = cst.tile([128, 1], F32)
            nc.scalar.activation(scratch[:], buf_f[:, 0:1], ACT.Square)

            s_bank = psum.tile([128, 512], F32, tag="S")
            dr_bank = psum.tile([128, 512], F32, tag="Dr")
            dc_bank = psum.tile([128, 512], F32, tag="Dc")
            U_LAST = NCH * NBLK - 1

            def mm(bank, src, u, cols=slice(0, 512), start=None, stop=None):
                nc.tensor.matmul(bank[:, cols], stat[:, 63 - u: 191 - u], src,
                                 start=(u == 0) if start is None else start,
                                 stop=False if stop is None else stop)

            # chunk 0 split into quarter/quarter/half input tiles so the
            # first squares (and the PE pipeline) start as early as possible
            til00 = cst.tile([128, QB], F32)
            til01 = cst.tile([128, QB], F32)
            til0r = cst.tile([128, HB], F32)
            sq00 = cst.tile([128, QB], F32R)
            sq01 = cst.tile([128, QB], F32R)
            sq0r = cst.tile([128, HB], F32R)
            # wrap tile for the tail boundary (B pixels [HALF, HALF+257))
            wrap = cst.tile([64, 257], F32)

            nc.sync.dma_start(til00[0:64, :], d_fm[:, 0: QB])
            nc.sync.dma_start(til00[64:128, :], d_fm[:, HALF: HALF + QB])
            nc.sync.dma_start(til01[0:64, :], d_fm[:, QB: 2 * QB])
            nc.sync.dma_start(til01[64:128, :], d_fm[:, HALF + QB: HALF + 2 * QB])
            nc.sync.dma_start(til0r[0:64, :], d_fm[:, HB: CH])
            nc.sync.dma_start(til0r[64:128, :], d_fm[:, HALF + HB: HALF + CH])
            nc.sync.dma_start(wrap[:], d_fm[:, HALF: HALF + 257])

            nc.scalar.activation(sq00[:], til00[:], ACT.Square)
            nc.scalar.activation(sq01[:], til01[:], ACT.Square)
            nc.scalar.activation(sq0r[:], til0r[:], ACT.Square)
            for s in range(2):
                mm(s_bank, sq00[:, 512 * s: 512 * (s + 1)], s, start=(s == 0))
            for s in range(2):
                mm(s_bank, sq01[:, 512 * s: 512 * (s + 1)], 2 + s)
            for s in range(4):
                mm(s_bank, sq0r[:, 512 * s: 512 * (s + 1)], 4 + s)

            # chunk-0 bodies across A=til00, B=til01, R=til0r
            prb0 = bod.tile([128, PRB_W], F32R, tag="prb")
            pcb0 = bod.tile([128, PCB_W], F32R, tag="pcb")
            nc.gpsimd.tensor_tensor(prb0[:, 0: QB - 256], til00[:, 0: QB - 256],
                                    til00[:, 256: QB], AL.mult)
            nc.vector.tensor_tensor(prb0[:, QB - 256: QB],
                                    til00[:, QB - 256: QB],
                                    til01[:, 0: 256], AL.mult)
            nc.gpsimd.tensor_tensor(prb0[:, QB: HB - 256], til01[:, 0: QB - 256],
                                    til01[:, 256: QB], AL.mult)
            nc.vector.tensor_tensor(prb0[:, HB - 256: HB],
                                    til01[:, QB - 256: QB],
                                    til0r[:, 0: 256], AL.mult)
            nc.gpsimd.tensor_tensor(prb0[:, HB: PRB_W], til0r[:, 0: PRB_W - HB],
                                    til0r[:, 256: 256 + PRB_W - HB], AL.mult)
            nc.vector.tensor_tensor(pcb0[:, 0: QB - 1], til00[:, 0: QB - 1],
                                    til00[:, 1: QB], AL.mult)
            nc.vector.tensor_tensor(pcb0[:, QB - 1: QB], til00[:, QB - 1: QB],
                                    til01[:, 0: 1], AL.mult)
            nc.vector.tensor_tensor(pcb0[:, QB: HB - 1], til01[:, 0: QB - 1],
                                    til01[:, 1: QB], AL.mult)
            nc.vector.tensor_tensor(pcb0[:, HB - 1: HB], til01[:, QB - 1: QB],
                                    til0r[:, 0: 1], AL.mult)
            nc.vector.tensor_tensor(pcb0[:, HB: PCB_W], til0r[:, 0: PCB_W - HB],
                                    til0r[:, 1: 1 + PCB_W - HB], AL.mult)
            # (pcb0 covers [0, 3840) like prb0; the tail 256 pc-pixels of the
            # chunk are handled by the boundary's pch256 head)
            for s in range(NBLK - 1):
                mm(dr_bank, prb0[:, 512 * s: 512 * (s + 1)], s, start=(s == 0))
                mm(dc_bank, pcb0[:, 512 * s: 512 * (s + 1)], s, start=(s == 0))

            tiles = {0: til0r}
            widths = {0: HB}
            prbs = {0: prb0}
            pcbs = {0: pcb0}

            def boundary(u, pb_prev, pc_prev, pv, pw, right256, right1):
                """Deferred matmuls for pixel block u (a chunk boundary):
                first 256/511 columns come from the previous chunk's extended
                bodies; the last columns are tiny head products computed from
                the previous input tile x the next chunk's first columns."""
                mm(dr_bank, pb_prev[:, PRB_W - 256: PRB_W], u, slice(0, 256))
                prh2 = hed.tile([128, 256], F32R, tag="prh")
                nc.vector.tensor_tensor(prh2[:], pv[:, pw - 256: pw],
                                        right256, AL.mult)
                mm(dr_bank, prh2[:], u, slice(256, 512))
                mm(dc_bank, pc_prev[:, PCB_W - 256: PCB_W], u, slice(0, 256))
                pch2 = hed.tile([128, 256], F32R, tag="pch")
                nc.vector.tensor_tensor(pch2[:, 0:255], pv[:, pw - 256: pw - 1],
                                        pv[:, pw - 255: pw], AL.mult)
                nc.vector.tensor_tensor(pch2[:, 255:256], pv[:, pw - 1: pw],
                                        right1, AL.mult)
                mm(dc_bank, pch2[:], u, slice(256, 512))

            for t in range(1, NCH - 1):
                a0 = t * CH
                b0 = HALF + t * CH
                tl = inp.tile([128, CH], F32, tag="in")
                nc.sync.dma_start(tl[0:64, :], d_fm[:, a0: a0 + CH])
                nc.sync.dma_start(tl[64:128, :], d_fm[:, b0: b0 + CH])

                boundary(t * NBLK - 1, prbs[t - 1], pcbs[t - 1],
                         tiles[t - 1], widths[t - 1],
                         tl[:, 0:256], tl[:, 0:1])

                # bodies (sq split in quarters for earlier matmul starts)
                sqq = []
                for i in range(4):
                    sq_q = bod.tile([128, QB], F32R, tag=f"sq{i}")
                    sqq.append(sq_q)
                prb = bod.tile([128, PRB_W], F32R, tag="prb")
                pcb = bod.tile([128, PCB_W], F32R, tag="pcb")
                for i in range(4):
                    nc.scalar.activation(sqq[i][:], tl[:, QB * i: QB * (i + 1)],
                                         ACT.Square)
                nc.gpsimd.tensor_tensor(prb[:, 0:POOL_COLS], tl[:, 0:POOL_COLS],
                                        tl[:, 256: 256 + POOL_COLS], AL.mult)
                nc.vector.tensor_tensor(prb[:, POOL_COLS:PRB_W],
                                        tl[:, POOL_COLS:PRB_W],
                                        tl[:, 256 + POOL_COLS: 256 + PRB_W],
                                        AL.mult)
                nc.vector.tensor_tensor(pcb[:], tl[:, 0:PCB_W],
                                        tl[:, 1: 1 + PCB_W], AL.mult)

                u0 = t * NBLK
                for s in range(NBLK):
                    mm(s_bank, sqq[s // 2][:, 512 * (s % 2): 512 * (s % 2 + 1)],
                       u0 + s)
                for s in range(NBLK - 1):
                    mm(dr_bank, prb[:, 512 * s: 512 * (s + 1)], u0 + s)
                    mm(dc_bank, pcb[:, 512 * s: 512 * (s + 1)], u0 + s)

                tiles[t] = tl
                widths[t] = CH
                prbs[t] = prb
                pcbs[t] = pcb

            # ---- last chunk, processed as two 2048-pixel halves ----
            TL = NCH - 1
            a0 = TL * CH
            b0 = HALF + TL * CH
            PRB_H = HB - 256   # 1792
            PCB_H = HB - 256   # 1792
            PHL = 1344         # Pool's share of each half-chunk prodrow body

            til7l = inp.tile([128, CH], F32, tag="in")
            nc.sync.dma_start(til7l[0:64, 0:HB], d_fm[:, a0: a0 + HB])
            nc.sync.dma_start(til7l[64:128, 0:HB], d_fm[:, b0: b0 + HB])

            boundary(TL * NBLK - 1, prbs[TL - 1], pcbs[TL - 1],
                     tiles[TL - 1], widths[TL - 1],
                     til7l[:, 0:256], til7l[:, 0:1])

            sq7la = bod.tile([128, QB], F32R, tag="sq0")
            sq7lb = bod.tile([128, QB], F32R, tag="sq1")
            prb7l = bod.tile([128, PRB_W], F32R, tag="prb")
            pcb7l = bod.tile([128, PCB_W], F32R, tag="pcb")
            nc.scalar.activation(sq7la[:], til7l[:, 0:QB], ACT.Square)
            nc.scalar.activation(sq7lb[:], til7l[:, QB:HB], ACT.Square)
            nc.gpsimd.tensor_tensor(prb7l[:, 0:PHL], til7l[:, 0:PHL],
                                    til7l[:, 256: 256 + PHL], AL.mult)
            nc.vector.tensor_tensor(prb7l[:, PHL:PRB_H], til7l[:, PHL:PRB_H],
                                    til7l[:, 256 + PHL: 256 + PRB_H], AL.mult)
            PCL = 512
            nc.gpsimd.tensor_tensor(pcb7l[:, 0:PCL], til7l[:, 0:PCL],
                                    til7l[:, 1: 1 + PCL], AL.mult)
            nc.vector.tensor_tensor(pcb7l[:, PCL:PCB_H], til7l[:, PCL:PCB_H],
                                    til7l[:, 1 + PCL: 1 + PCB_H], AL.mult)
            u0 = TL * NBLK
            for s in range(4):
                sq_h = sq7la if s < 2 else sq7lb
                mm(s_bank, sq_h[:, 512 * (s % 2): 512 * (s % 2 + 1)], u0 + s)
            for s in range(3):
                mm(dr_bank, prb7l[:, 512 * s: 512 * (s + 1)], u0 + s)
                mm(dc_bank, pcb7l[:, 512 * s: 512 * (s + 1)], u0 + s)

            til7r = inp.tile([128, CH], F32, tag="in")
            nc.sync.dma_start(til7r[0:64, 0:HB], d_fm[:, a0 + HB: a0 + CH])
            nc.sync.dma_start(til7r[64:128, 0:HB], d_fm[:, b0 + HB: b0 + CH])

            # boundary b7.5 (block u0+3) between the 7L and 7R halves
            mm(dr_bank, prb7l[:, PRB_H - 256: PRB_H], u0 + 3, slice(0, 256))
            prh2m = hed.tile([128, 256], F32R, tag="prh")
            nc.vector.tensor_tensor(prh2m[:], til7l[:, HB - 256: HB],
                                    til7r[:, 0:256], AL.mult)
            mm(dr_bank, prh2m[:], u0 + 3, slice(256, 512))
            mm(dc_bank, pcb7l[:, PCB_H - 256: PCB_H], u0 + 3, slice(0, 256))
            pch2m = hed.tile([128, 256], F32R, tag="pch")
            nc.vector.tensor_tensor(pch2m[:, 0:255], til7l[:, HB - 256: HB - 1],
                                    til7l[:, HB - 255: HB], AL.mult)
            nc.vector.tensor_tensor(pch2m[:, 255:256], til7l[:, HB - 1: HB],
                                    til7r[:, 0:1], AL.mult)
            mm(dc_bank, pch2m[:], u0 + 3, slice(256, 512))

            # bodies 7R
            sq7ra = bod.tile([128, QB], F32R, tag="sq2")
            sq7rb = bod.tile([128, QB], F32R, tag="sq3")
            prb7r = bod.tile([128, PRB_W], F32R, tag="prb")
            pcb7r = bod.tile([128, PCB_W], F32R, tag="pcb")
            nc.scalar.activation(sq7ra[:], til7r[:, 0:QB], ACT.Square)
            nc.scalar.activation(sq7rb[:], til7r[:, QB:HB], ACT.Square)
            nc.gpsimd.tensor_tensor(prb7r[:, 0:PHL], til7r[:, 0:PHL],
                                    til7r[:, 256: 256 + PHL], AL.mult)
            nc.vector.tensor_tensor(prb7r[:, PHL:PRB_H], til7r[:, PHL:PRB_H],
                                    til7r[:, 256 + PHL: 256 + PRB_H], AL.mult)
            nc.gpsimd.tensor_tensor(pcb7r[:, 0:PCL], til7r[:, 0:PCL],
                                    til7r[:, 1: 1 + PCL], AL.mult)
            nc.vector.tensor_tensor(pcb7r[:, PCL:PCB_H], til7r[:, PCL:PCB_H],
                                    til7r[:, 1 + PCL: 1 + PCB_H], AL.mult)
            u1 = u0 + 4
            for s in range(4):
                sq_h = sq7ra if s < 2 else sq7rb
                mm(s_bank, sq_h[:, 512 * (s % 2): 512 * (s % 2 + 1)], u1 + s,
                   stop=(s == 3))

            # tail boundary b8 (block 63): A rows wrap to B pixels
            # [HALF, HALF+256); B rows (image row 255) have no row edge ->
            # finite garbage via self-products.
            mm(dr_bank, prb7r[:, PRB_H - 256: PRB_H], U_LAST, slice(0, 256))
            prt2 = hed.tile([128, 256], F32R, tag="prh")
            nc.vector.tensor_tensor(prt2[0:64, :], til7r[0:64, HB - 256: HB],
                                    wrap[:, 0:256], AL.mult)
            nc.vector.tensor_tensor(prt2[64:128, :], til7r[64:128, HB - 256: HB],
                                    til7r[64:128, HB - 256: HB], AL.mult)
            mm(dr_bank, prt2[:], U_LAST, slice(256, 512))
            mm(dc_bank, pcb7r[:, PCB_H - 256: PCB_H], U_LAST, slice(0, 256))
            pct2 = hed.tile([128, 256], F32R, tag="pch")
            nc.vector.tensor_tensor(pct2[:, 0:255], til7r[:, HB - 256: HB - 1],
                                    til7r[:, HB - 255: HB], AL.mult)
            nc.vector.tensor_tensor(pct2[0:64, 255:256], til7r[0:64, HB - 1: HB],
                                    wrap[:, 0:1], AL.mult)
            nc.vector.tensor_tensor(pct2[64:128, 255:256],
                                    til7r[64:128, HB - 1: HB],
                                    til7r[64:128, HB - 1: HB], AL.mult)
            mm(dc_bank, pct2[:], U_LAST, slice(256, 512))

            for s in range(3):
                mm(dr_bank, prb7r[:, 512 * s: 512 * (s + 1)], u1 + s,
                   stop=(s == 2))
                mm(dc_bank, pcb7r[:, 512 * s: 512 * (s + 1)], u1 + s,
                   stop=(s == 2))

            # ---- finalize ----
            # S-dependent prefix first (the S bank completes with the 7R sq
            # matmuls, before the Dr/Dc banks stop)
            s_lo = fin.tile([128, 256], F32)
            s_hi = fin.tile([128, 256], F32)
            nc.scalar.copy(s_lo[:], s_bank[:, 0:256])
            nc.scalar.copy(s_hi[:], s_bank[:, 256:512])
            sdn = fin.tile([128, 256], F32)
            nc.vector.memset(sdn[:], 0.0)
            nc.sync.dma_start(sdn[0:127, :], s_lo[1:128, :])

            tmp = fin.tile([128, 512], F32)
            tmq = fin.tile([128, 512], F32)
            wrow = fin.tile([128, 512], F32)
            wcol = fin.tile([128, 512], F32)
            nc.gpsimd.memset(wcol[:, 511:512], 1.0)
            # wcol helper: (S[u,f]+1) + S[u,f+1]
            nc.vector.scalar_tensor_tensor(tmq[:, 0:255], s_lo[:, 0:255], 1.0,
                                           s_lo[:, 1:256], AL.add, AL.add)
            nc.vector.scalar_tensor_tensor(tmq[:, 255:256], s_lo[:, 255:256],
                                           1.0, s_hi[:, 0:1], AL.add, AL.add)
            nc.vector.scalar_tensor_tensor(tmq[:, 256:511], s_hi[:, 0:255], 1.0,
                                           s_hi[:, 1:256], AL.add, AL.add)
            # wrow helpers: (S[u,f]+1) + S[u,f+256] / + S[u+1,f-256]
            nc.vector.scalar_tensor_tensor(tmp[:, 0:256], s_lo[:], 1.0,
                                           s_hi[:], AL.add, AL.add)
            # tmp_hi = S_hi + 1 (sdn joins only in the last add below)
            nc.vector.tensor_scalar_add(tmp[:, 256:512], s_hi[:], 1.0)

            # final combines; wcol finishes first so its single output DMA
            # overlaps the wrow combines
            nc.vector.scalar_tensor_tensor(wcol[:, 0:511], dc_bank[:, 0:511],
                                           -2.0, tmq[:, 0:511], AL.mult, AL.add)
            nc.sync.dma_start(o_wcol[:], wcol[:])
            nc.vector.scalar_tensor_tensor(wrow[:, 0:256], dr_bank[:, 0:256],
                                           -2.0, tmp[:, 0:256], AL.mult, AL.add)
            nc.sync.dma_start(o_wrow[:, 0:256], wrow[:, 0:256])
            pre = fin.tile([128, 256], F32)
            nc.vector.scalar_tensor_tensor(pre[:], dr_bank[:, 256:512],
                                           -2.0, tmp[:, 256:512], AL.mult, AL.add)
            nc.vector.tensor_tensor(wrow[:, 256:512], pre[:], sdn[:], AL.add)
            nc.sync.dma_start(o_wrow[:, 256:512], wrow[:, 256:512])

    nc.compile()
    return nc


def _get_program():
    global _compiled
    if _compiled is None:
        _compiled = _build_program()
    return _compiled


def _edge_weights_device(guide_in):
    """Run the bass program on 8 cores; returns (wr [B,255,256], wc [B,256,255])."""
    from concourse.bass_utils import run_bass_kernel_spmd

    nc = _get_program()
    in_maps = [{"fm": np.ascontiguousarray(guide_in[b].reshape(C, V))}
               for b in range(B)]
    res = run_bass_kernel_spmd(nc, in_maps, list(range(8)))

    wr, wc = [], []
    for b in range(B):
        r = res.results[b]
        wrow = np.asarray(r["wrow"]).reshape(H, W)
        wcol = np.asarray(r["wcol"]).reshape(H, W)
        wr.append(wrow[:H - 1, :])
        wc.append(wcol[:, :W - 1])
    return np.stack(wr), np.stack(wc)


def _build_index():
    raw = np.arange(V, dtype=np.int32).reshape(H, W)
    row_e = np.stack([raw[:-1, :], raw[1:, :]], axis=-1).reshape(-1, 2)
    col_e = np.stack([raw[:, :-1], raw[:, 1:]], axis=-1).reshape(-1, 2)
    return np.concatenate([row_e, col_e], axis=0)


def _scatter_min(target, keys, vals):
    order = np.argsort(keys, kind="stable")
    ks = keys[order]
    vs = vals[order]
    starts = np.flatnonzero(np.r_[True, ks[1:] != ks[:-1]])
    mins = np.minimum.reduceat(vs, starts)
    target[ks[starts]] = np.minimum(target[ks[starts]], mins)


def _mst_boruvka(u, v, w):
    """Exact port of the reference Boruvka (per image)."""
    eidx = np.arange(E, dtype=np.int64)
    vidx = np.arange(V, dtype=np.int64)
    INF = np.float32(np.inf)
    BIGE = E
    comp = vidx.copy()
    sel = np.zeros(E, dtype=bool)
    for _ in range(N_ROUNDS):
        cu, cv = comp[u], comp[v]
        active = cu != cv
        if not active.any():
            break
        wa = np.where(active, w, INF)
        minw = np.full(V, INF, np.float32)
        _scatter_min(minw, cu, wa)
        _scatter_min(minw, cv, wa)
        cand_u = np.where(active & (wa == minw[cu]), eidx, BIGE)
        cand_v = np.where(active & (wa == minw[cv]), eidx, BIGE)
        best = np.full(V, BIGE, np.int64)
        _scatter_min(best, cu, cand_u)
        _scatter_min(best, cv, cand_v)
        has = best < BIGE
        be = np.clip(best, 0, E - 1)
        cu_b, cv_b = comp[u[be]], comp[v[be]]
        parent = np.where(has, np.where(cu_b == vidx, cv_b, cu_b), vidx)
        pp = parent[parent]
        parent = np.where((pp == vidx) & (vidx < parent), vidx, parent)
        for _ in range(N_ROUNDS):
            parent = parent[parent]
        comp = parent[comp]
        sel_idx = best[has]
        sel[sel_idx] = True
    return sel


def kernel(guide_in):
    guide_in = np.asarray(guide_in, dtype=np.float32)
    wr, wc = _edge_weights_device(guide_in)

    index = _build_index()
    u = index[:, 0].astype(np.int64)
    v = index[:, 1].astype(np.int64)
    trees = []
    for b in range(B):
        w = np.concatenate([wr[b].reshape(-1), wc[b].reshape(-1)]).astype(np.float32)
        sel = _mst_boruvka(u, v, w)
        eids = np.nonzero(sel)[0]
        if len(eids) != V - 1:  # pad/trim defensively (should be exactly V-1)
            eids = np.concatenate([eids, np.zeros(max(0, V - 1 - len(eids)), np.int64)])[:V - 1]
        trees.append(index[eids])
    return np.stack(trees).astype(np.int32)

